# revision 25
# baseline (speedup 1.0000x reference)
"""Trainium2 Bass kernel for nn_CPPN: 3-layer MLP (4->64->64->3, tanh) over
1M pixels + global min/max normalization, data-parallel over 8 NeuronCores.

Layout strategy (per core, NPIX = 131072 pixels):
  - pixels are split into 32 "subsets" of 4096 contiguous pixels each.
  - subset s lives at partitions 32*g + 4*q + i (g = s%4 row-group,
    q = s//4, i = input feature), so layer-1 runs as K=32 matmuls with
    zero-padded weights, 4 row-groups concurrently on the PE array.
  - hidden states keep features on partitions ([64|64] per [128, 1024]
    tile = 4 subsets), so tanh runs on full 128-lane ACT ops straight out
    of 2-bank PSUM tiles with the bias fused into the activation.
  - layer-3 uses a block-diagonal [128, 6] weight so each matmul emits two
    subsets' 3 channels densely, packed 4-per-PSUM-bank via column slots.
  - global min/max: per-partition running min/max on DVE, one tiny
    AllReduce(max) of (-min, max) across the 8 cores, then a second
    normalize+clip pass over the staged output.
"""

import os
import numpy as np

B, N, NI, H, NO = 4, 262144, 4, 64, 3
NCORES = 8
NPIX_TOT = B * N
NPIX = NPIX_TOT // NCORES      # 131072 pixels per core
NSUB = 32                      # subsets per core
SUBPIX = NPIX // NSUB          # 4096 pixels per subset
CW = 512                       # matmul moving-dim chunk width
NCHUNK = SUBPIX // CW          # 8 chunks
F32MAX = 3.0e38

_CACHE = {}
LAST_RESULTS = None            # test.py reads exec_time_ns from here


def _build_module(mm_dtype_name="bfloat16"):
    import concourse.bass as bass
    import concourse.tile as tile
    from concourse import bacc, mybir
    from concourse.tile import add_dep_helper

    dt = mybir.dt
    alu = mybir.AluOpType
    act = mybir.ActivationFunctionType
    f32 = dt.float32
    mmdt = getattr(dt, mm_dtype_name)

    nc = bacc.Bacc("TRN2", target_bir_lowering=False, debug=False,
                   num_devices=NCORES)

    x_d = nc.dram_tensor("xcore", [128, SUBPIX], mmdt, kind="ExternalInput").ap()
    w1_d = nc.dram_tensor("w1s", [128, 8 * H], mmdt, kind="ExternalInput").ap()
    w2_d = nc.dram_tensor("w2s", [128, H], mmdt, kind="ExternalInput").ap()
    w3_d = nc.dram_tensor("w3bd", [128, 32], mmdt, kind="ExternalInput").ap()
    b1_d = nc.dram_tensor("b1s", [128, 1], f32, kind="ExternalInput").ap()
    b2_d = nc.dram_tensor("b2s", [128, 1], f32, kind="ExternalInput").ap()
    b3_d = nc.dram_tensor("b3s", [128, 1], f32, kind="ExternalInput").ap()
    out_d = nc.dram_tensor("out", [NO, NPIX], f32, kind="ExternalOutput").ap()

    cc_in = nc.dram_tensor("cc_in", [48], f32).ap()
    cc_out = nc.dram_tensor("cc_out", [48], f32, addr_space="Shared").ap()

    with tile.TileContext(nc) as tc:
        with tc.tile_pool(name="const", bufs=1) as const, \
             tc.tile_pool(name="stage", bufs=1) as stage, \
             tc.tile_pool(name="hid", bufs=3) as hid, \
             tc.tile_pool(name="mm", bufs=1) as mmp, \
             tc.tile_pool(name="ps1", bufs=2, space="PSUM") as ps1, \
             tc.tile_pool(name="ps2", bufs=1, space="PSUM") as ps2, \
             tc.tile_pool(name="ps3", bufs=2, space="PSUM") as ps3:

            xin = const.tile([128, SUBPIX], mmdt, tag="xin")
            w1s = const.tile([128, 8 * H], mmdt, tag="w1s")
            w2s = const.tile([128, H], mmdt, tag="w2s")
            w3bd = const.tile([128, 32], mmdt, tag="w3bd")
            b1s = const.tile([128, 1], f32, tag="b1s")
            b2s = const.tile([128, 1], f32, tag="b2s")
            b3s = const.tile([128, 1], f32, tag="b3s")

            nc.sync.dma_start(out=xin[:], in_=x_d)
            nc.sync.dma_start(out=w1s[:], in_=w1_d)
            nc.sync.dma_start(out=w2s[:], in_=w2_d)
            nc.sync.dma_start(out=w3bd[:], in_=w3_d)
            nc.sync.dma_start(out=b1s[:], in_=b1_d)
            nc.sync.dma_start(out=b2s[:], in_=b2_d)
            nc.sync.dma_start(out=b3s[:], in_=b3_d)

            # staged (pre-norm) output, [128, NCHUNK*2048] sparse layout:
            # partition 32w + 3a + o, free c*2048 + 512*b + n
            ostage = stage.tile([128, NCHUNK * 2048], f32, tag="ostage")

            rmin = mmp.tile([128, 1], f32, tag="rmin")
            rmax = mmp.tile([128, 1], f32, tag="rmax")
            nc.vector.memset(rmin[:], F32MAX)
            nc.vector.memset(rmax[:], -F32MAX)

            for c in range(NCHUNK):
                ps3t = None
                for t in range(8):          # pair-tile: subsets 4t..4t+3
                    p1 = ps1.tile([128, 2 * CW], f32, tag="p1")
                    for v in range(2):
                        for a in range(2):
                            s = 4 * t + 2 * v + a
                            g, q = s % 4, s // 4
                            nc.tensor.matmul(
                                out=p1[64 * a: 64 * a + 64, CW * v: CW * v + CW],
                                lhsT=w1s[32 * g: 32 * g + 32,
                                         H * q: H * q + H],
                                rhs=xin[32 * g: 32 * g + 32,
                                        c * CW: (c + 1) * CW],
                                start=True, stop=True,
                                tile_position=(32 * g, 64 * a))
                    h1 = hid.tile([128, 2 * CW], mmdt, tag="h1")
                    nc.scalar.activation(h1[:], p1[:], act.Tanh, bias=b1s[:])

                    p2 = ps2.tile([128, 2 * CW], f32, tag="p2")
                    for v in range(2):
                        for a in range(2):
                            nc.tensor.matmul(
                                out=p2[64 * a: 64 * a + 64, CW * v: CW * v + CW],
                                lhsT=w2s[64 * a: 64 * a + 64, :],
                                rhs=h1[64 * a: 64 * a + 64,
                                       CW * v: CW * v + CW],
                                start=True, stop=True,
                                tile_position=(64 * a, 64 * a))
                    h2 = hid.tile([128, 2 * CW], mmdt, tag="h2")
                    nc.scalar.activation(h2[:], p2[:], act.Tanh, bias=b2s[:])

                    for v in range(2):
                        u = 2 * t + v       # 0..15 within chunk
                        bk, w = u // 4, u % 4
                        if w == 0:
                            ps3t = ps3.tile([128, CW], f32, tag="p3")
                        nc.tensor.matmul(
                            out=ps3t[32 * w: 32 * w + 32, :],
                            lhsT=w3bd[:],
                            rhs=h2[:, CW * v: CW * v + CW],
                            start=True, stop=True,
                            tile_position=(0, 32 * w))
                        if w == 3:
                            # bank full -> evacuate with bias fused
                            nc.vector.tensor_scalar(
                                ostage[:, c * 2048 + CW * bk:
                                       c * 2048 + CW * bk + CW],
                                ps3t[:], b3s[:], None, alu.add)

                oc = ostage[:, c * 2048: (c + 1) * 2048]
                cmin = mmp.tile([128, 1], f32, tag="cmin")
                cmax = mmp.tile([128, 1], f32, tag="cmax")
                nc.vector.tensor_reduce(cmin[:], oc, mybir.AxisListType.X, alu.min)
                nc.vector.tensor_reduce(cmax[:], oc, mybir.AxisListType.X, alu.max)
                nc.vector.tensor_tensor(rmin[:], rmin[:], cmin[:], alu.min)
                nc.vector.tensor_tensor(rmax[:], rmax[:], cmax[:], alu.max)

            # pack (-min, max) and gather the 24 valid partitions to DRAM
            mmt = mmp.tile([128, 2], f32, tag="mmt")
            nc.vector.tensor_scalar(mmt[:, 0:1], rmin[:], -1.0, None, alu.mult)
            nc.vector.tensor_copy(mmt[:, 1:2], rmax[:])
            cc_in_v = cc_in.rearrange("(w r x) -> w r x", w=4, x=2)
            gather_dmas = []
            for w in range(4):
                d = nc.sync.dma_start(out=cc_in_v[w],
                                      in_=mmt[32 * w: 32 * w + 6, :])
                gather_dmas.append(d)

            coll = nc.gpsimd.collective_compute(
                "AllReduce", alu.max,
                replica_groups=[list(range(NCORES))],
                ins=[cc_in], outs=[cc_out])
            for d in gather_dmas:
                add_dep_helper(coll.ins, d.ins, reason="gather before allreduce")

            scb = mmp.tile([128, 48], f32, tag="scb")
            bd = nc.sync.dma_start(out=scb[:], in_=cc_out.partition_broadcast(128))
            add_dep_helper(bd.ins, coll.ins, reason="bcast after allreduce")

            scb_v = scb[:].rearrange("p (k x) -> p k x", x=2)
            nmin = mmp.tile([128, 1], f32, tag="nmin")
            gmax = mmp.tile([128, 1], f32, tag="gmax")
            nc.vector.tensor_reduce(nmin[:], scb_v[:, :, 0],
                                    mybir.AxisListType.X, alu.max)
            nc.vector.tensor_reduce(gmax[:], scb_v[:, :, 1],
                                    mybir.AxisListType.X, alu.max)
            rng = mmp.tile([128, 1], f32, tag="rng")
            nc.vector.tensor_tensor(rng[:], gmax[:], nmin[:], alu.add)
            inv = mmp.tile([128, 1], f32, tag="inv")
            nc.vector.reciprocal(inv[:], rng[:])
            off = mmp.tile([128, 1], f32, tag="off")
            nc.vector.tensor_tensor(off[:], nmin[:], inv[:], alu.mult)

            out_v = out_d.rearrange("o (b r c n) -> o b r c n",
                                    b=4, r=8, c=NCHUNK)
            for c in range(NCHUNK):
                oc = ostage[:, c * 2048: (c + 1) * 2048]
                nc.vector.tensor_scalar(oc, oc, inv[:], off[:],
                                        alu.mult, alu.add)
                nc.vector.tensor_scalar(oc, oc, 0.0, 1.0, alu.max, alu.min)
                for w in range(4):
                    for a in range(2):
                        sb = ostage[32 * w + 3 * a: 32 * w + 3 * a + 3,
                                    c * 2048: (c + 1) * 2048]
                        sb = sb.rearrange("p (b n) -> p b n", n=CW)
                        nc.sync.dma_start(out=out_v[:, :, 2 * w + a, c, :],
                                          in_=sb)
    nc.compile()
    return nc


def _host_inputs(x, W1, b1, W2, b2, W3, b3, mm_np=None):
    """Repack full inputs into per-core in_maps (host-side, not HW-timed)."""
    if mm_np is None:
        import ml_dtypes
        mm = os.environ.get("CPPN_MM_DTYPE", "bfloat16")
        mm_np = ml_dtypes.bfloat16 if mm == "bfloat16" else np.float32
    x = np.asarray(x, np.float32).reshape(NPIX_TOT, NI)
    W1 = np.asarray(W1, np.float32)
    b1 = np.asarray(b1, np.float32)
    W2 = np.asarray(W2, np.float32)
    b2 = np.asarray(b2, np.float32)
    W3 = np.asarray(W3, np.float32)
    b3 = np.asarray(b3, np.float32)

    blk = np.zeros((32, 8 * H), np.float32)
    for q in range(8):
        blk[4 * q: 4 * q + 4, H * q: H * q + H] = W1
    w1s = np.tile(blk, (4, 1))

    w2s = np.concatenate([W2, W2], axis=0)
    w3bd = np.zeros((128, 32), np.float32)
    w3bd[0:64, 0:3] = W3
    w3bd[64:128, 3:6] = W3

    b1s = np.concatenate([b1, b1])[:, None].astype(np.float32)
    b2s = np.concatenate([b2, b2])[:, None].astype(np.float32)
    b3blk = np.zeros(32, np.float32)
    b3blk[0:3] = b3
    b3blk[3:6] = b3
    b3s = np.tile(b3blk, 4)[:, None].astype(np.float32)

    in_maps = []
    for k in range(NCORES):
        shard = x[k * NPIX: (k + 1) * NPIX].reshape(NSUB, SUBPIX, NI)
        xcore = np.empty((128, SUBPIX), np.float32)
        for s in range(NSUB):
            g, q = s % 4, s // 4
            xcore[32 * g + 4 * q: 32 * g + 4 * q + 4, :] = shard[s].T
        in_maps.append({
            "xcore": np.ascontiguousarray(xcore).astype(mm_np),
            "w1s": w1s.astype(mm_np), "w2s": w2s.astype(mm_np),
            "w3bd": w3bd.astype(mm_np),
            "b1s": b1s, "b2s": b2s, "b3s": b3s,
        })
    return in_maps


def kernel(x, W1, b1, W2, b2, W3, b3):
    global LAST_RESULTS
    from concourse.bass_utils import run_bass_kernel_spmd

    mm = os.environ.get("CPPN_MM_DTYPE", "bfloat16")
    if mm not in _CACHE:
        _CACHE[mm] = _build_module(mm)
    nc = _CACHE[mm]

    in_maps = _host_inputs(x, W1, b1, W2, b2, W3, b3)
    res = run_bass_kernel_spmd(nc, in_maps, list(range(NCORES)))
    LAST_RESULTS = res

    out = np.empty((NO, NPIX_TOT), np.float32)
    for k in range(NCORES):
        out[:, k * NPIX: (k + 1) * NPIX] = res.results[k]["out"]
    return out.reshape(NO, B, N)


# revision 29
# speedup vs baseline: 1.0424x; 1.0424x over previous
"""Trainium2 Bass kernel for nn_CPPN: 3-layer MLP (4->64->64->3, tanh) over
1M pixels + global min/max normalization, data-parallel over 8 NeuronCores.

Layout strategy (per core, NPIX = 131072 pixels):
  - pixels split into 32 "subsets" of 4096 contiguous pixels; subset s lives
    at partitions 32*(s%4) + 4*(s//4) + i (i = input feature), so layer-1
    runs as K=32 matmuls with zero-padded weights, 4 row-groups concurrent.
  - hidden states keep features on partitions ([64|64] per [128, 1024] tile
    = 4 subsets); tanh runs as full-128-lane ACT ops straight out of 2-bank
    PSUM tiles with the bias fused into the activation.
  - layer-2 packs 4 concurrent 64x64 matmuls via (row, col) tile positions,
    swapping output halves on odd column-halves (undone at host unshard).
  - layer-3 uses a block-diagonal [128, 32] weight emitting two subsets' 3
    channels at partition offsets {0,1,2,16,17,18} of a 32-slot, 4 slots
    per PSUM bank; DVE evacuates with fused bias.
  - ACT is software-pipelined (tanh2 of pair t emitted after tanh1 of
    t+1) with a single shared 3-slot PSUM pool so tanh runs back-to-back.
  - global min/max: per-partition running min/max on DVE, gpsimd
    cross-partition reduce, one tiny AllGather, then per-chunk
    normalize+clip+DMA-out interleaved.
"""

import os
import numpy as np

B, N, NI, H, NO = 4, 262144, 4, 64, 3
NCORES = 8
NPIX_TOT = B * N
NPIX = NPIX_TOT // NCORES      # 131072 pixels per core
NSUB = 32                      # subsets per core
SUBPIX = NPIX // NSUB          # 4096 pixels per subset
CW = 512                       # matmul moving-dim chunk width
NCHUNK = SUBPIX // CW          # 8 chunks
NT = NCHUNK * 8                # 64 pair-tiles total (8 per chunk)
F32MAX = 3.0e38

_CACHE = {}
LAST_RESULTS = None            # test.py reads exec_time_ns from here


def _build_module(mm_dtype_name="bfloat16"):
    import concourse.bass as bass
    import concourse.tile as tile
    from concourse import bacc, mybir
    from concourse.tile import add_dep_helper

    dt = mybir.dt
    alu = mybir.AluOpType
    act = mybir.ActivationFunctionType
    f32 = dt.float32
    mmdt = getattr(dt, mm_dtype_name)

    nc = bacc.Bacc("TRN2", target_bir_lowering=False, debug=False,
                   num_devices=NCORES)

    x_d = nc.dram_tensor("xcore", [128, SUBPIX], mmdt, kind="ExternalInput").ap()
    w1_d = nc.dram_tensor("w1s", [128, 8 * H], mmdt, kind="ExternalInput").ap()
    w2_d = nc.dram_tensor("w2s", [128, H], mmdt, kind="ExternalInput").ap()
    w3_d = nc.dram_tensor("w3bd", [128, 32], mmdt, kind="ExternalInput").ap()
    b1_d = nc.dram_tensor("b1s", [128, 1], f32, kind="ExternalInput").ap()
    b2_d = nc.dram_tensor("b2s", [128, 1], f32, kind="ExternalInput").ap()
    b3_d = nc.dram_tensor("b3s", [128, 1], f32, kind="ExternalInput").ap()
    vm_d = nc.dram_tensor("validm", [128, 1], f32, kind="ExternalInput").ap()
    nb_d = nc.dram_tensor("negb", [128, 1], f32, kind="ExternalInput").ap()
    out_d = nc.dram_tensor("out", [24, NCHUNK * 2048], f32,
                           kind="ExternalOutput").ap()

    cc_in = nc.dram_tensor("cc_in", [2], f32).ap()
    cc_out = nc.dram_tensor("cc_out", [2 * NCORES], f32,
                            addr_space="Shared").ap()

    with tile.TileContext(nc) as tc:
        with tc.tile_pool(name="const", bufs=1) as const, \
             tc.tile_pool(name="stage", bufs=1) as stage, \
             tc.tile_pool(name="hid", bufs=2) as hid, \
             tc.tile_pool(name="mm", bufs=1) as mmp, \
             tc.tile_pool(name="pmm", bufs=3, space="PSUM") as pmm, \
             tc.tile_pool(name="ps3", bufs=2, space="PSUM") as ps3:

            xin = const.tile([128, SUBPIX], mmdt, tag="xin")
            w1s = const.tile([128, 8 * H], mmdt, tag="w1s")
            w2s = const.tile([128, H], mmdt, tag="w2s")
            w3bd = const.tile([128, 32], mmdt, tag="w3bd")
            b1s = const.tile([128, 1], f32, tag="b1s")
            b2s = const.tile([128, 1], f32, tag="b2s")
            b3s = const.tile([128, 1], f32, tag="b3s")
            vms = const.tile([128, 1], f32, tag="vms")
            nbs = const.tile([128, 1], f32, tag="nbs")

            nc.sync.dma_start(out=w1s[:], in_=w1_d)
            nc.sync.dma_start(out=w2s[:], in_=w2_d)
            nc.sync.dma_start(out=w3bd[:], in_=w3_d)
            nc.sync.dma_start(out=b1s[:], in_=b1_d)
            nc.sync.dma_start(out=b2s[:], in_=b2_d)
            nc.sync.dma_start(out=b3s[:], in_=b3_d)
            nc.sync.dma_start(out=vms[:], in_=vm_d)
            nc.sync.dma_start(out=nbs[:], in_=nb_d)
            for c in range(NCHUNK):
                nc.sync.dma_start(out=xin[:, c * CW: (c + 1) * CW],
                                  in_=x_d[:, c * CW: (c + 1) * CW])

            # staged pre-norm output: partition 32w + 16a + o,
            # free c*2048 + 512*b + n; (w, a, b) identify subsets.
            ostage = stage.tile([128, NCHUNK * 2048], f32, tag="ostage")

            rmin = mmp.tile([128, 1], f32, tag="rmin")
            rmax = mmp.tile([128, 1], f32, tag="rmax")
            nc.vector.memset(rmin[:], F32MAX)
            nc.vector.memset(rmax[:], -F32MAX)

            def emit_l1_tanh1(t):
                c = t // 8
                p1 = pmm.tile([128, 2 * CW], f32, tag="pmm")
                for v in range(2):
                    for a in range(2):
                        s = 4 * (t % 8) + 2 * v + a
                        g, q = s % 4, s // 4
                        nc.tensor.matmul(
                            out=p1[64 * a: 64 * a + 64, CW * v: CW * v + CW],
                            lhsT=w1s[32 * g: 32 * g + 32, H * q: H * q + H],
                            rhs=xin[32 * g: 32 * g + 32,
                                    c * CW: (c + 1) * CW],
                            start=True, stop=True,
                            tile_position=(32 * g, 64 * a))
                h1 = hid.tile([128, 2 * CW], mmdt, tag="h1")
                nc.scalar.activation(h1[:], p1[:], act.Tanh, bias=b1s[:])
                return h1

            def emit_l2(t, h1):
                p2 = pmm.tile([128, 2 * CW], f32, tag="pmm")
                for v in range(2):
                    for a in range(2):
                        # odd column-half swaps output halves so all four
                        # matmuls pack onto disjoint PE subarray quadrants
                        ao = a ^ (v & 1)
                        nc.tensor.matmul(
                            out=p2[64 * ao: 64 * ao + 64,
                                   CW * v: CW * v + CW],
                            lhsT=w2s[64 * a: 64 * a + 64, :],
                            rhs=h1[64 * a: 64 * a + 64,
                                   CW * v: CW * v + CW],
                            start=True, stop=True,
                            tile_position=(64 * a, 64 * ao))
                return p2

            def emit_tanh2(t, p2):
                h2 = hid.tile([128, 2 * CW], mmdt, tag="h2")
                nc.scalar.activation(h2[:], p2[:], act.Tanh, bias=b2s[:])
                return h2

            ps3_box = [None]

            def emit_l3(t, h2):
                c = t // 8
                for v in range(2):
                    u = 2 * t + v
                    w = u % 4
                    if w == 0:
                        ps3_box[0] = ps3.tile([128, CW], f32, tag="p3",
                                              name=f"p3t{u}")
                    p3 = ps3_box[0]
                    nc.tensor.matmul(
                        out=p3[32 * w: 32 * w + 32, :],
                        lhsT=w3bd[:],
                        rhs=h2[:, CW * v: CW * v + CW],
                        start=True, stop=True,
                        tile_position=(0, 32 * w))
                    if w == 3:
                        bkk = (u // 4) % 4
                        nc.vector.tensor_scalar(
                            ostage[:, c * 2048 + CW * bkk:
                                   c * 2048 + CW * bkk + CW],
                            p3[:], b3s[:], None, alu.add)

            def emit_chunk_minmax(c):
                oc = ostage[:, c * 2048: (c + 1) * 2048]
                cmin = mmp.tile([128, 1], f32, tag="cmin")
                cmax = mmp.tile([128, 1], f32, tag="cmax")
                nc.vector.tensor_reduce(cmin[:], oc, mybir.AxisListType.X,
                                        alu.min)
                nc.vector.tensor_reduce(cmax[:], oc, mybir.AxisListType.X,
                                        alu.max)
                nc.vector.tensor_tensor(rmin[:], rmin[:], cmin[:], alu.min)
                nc.vector.tensor_tensor(rmax[:], rmax[:], cmax[:], alu.max)

            # ---- software-pipelined main loop (ACT lag-1 interleave) ----
            pending = {}
            for t in range(NT + 1):
                if t < NT:
                    h1 = emit_l1_tanh1(t)
                    pending[t] = emit_l2(t, h1)
                if t - 1 >= 0:
                    tp = t - 1
                    h2 = emit_tanh2(tp, pending.pop(tp))
                    emit_l3(tp, h2)
                    if tp % 8 == 7:
                        emit_chunk_minmax(tp // 8)

            # ---- global min/max via AllGather ----
            mmt = mmp.tile([128, 2], f32, tag="mmt")
            nc.vector.tensor_scalar(mmt[:, 0:1], rmin[:], -1.0, None, alu.mult)
            nc.vector.tensor_copy(mmt[:, 1:2], rmax[:])
            # mask garbage partitions to -inf so they never win the max
            nc.vector.tensor_scalar(mmt[:], mmt[:], vms[:], nbs[:],
                                    alu.mult, alu.add)
            red = mmp.tile([1, 2], f32, tag="red")
            nc.gpsimd.tensor_reduce(red[:], mmt[:], mybir.AxisListType.C,
                                    alu.max)
            gd = nc.sync.dma_start(out=cc_in, in_=red[:])
            coll = nc.gpsimd.collective_compute(
                "AllGather", alu.bypass,
                replica_groups=[list(range(NCORES))],
                ins=[cc_in], outs=[cc_out])
            add_dep_helper(coll.ins, gd.ins, reason="gather before allgather")
            scb = mmp.tile([128, 2 * NCORES], f32, tag="scb")
            bd = nc.sync.dma_start(out=scb[:],
                                   in_=cc_out.partition_broadcast(128))
            add_dep_helper(bd.ins, coll.ins, reason="bcast after allgather")

            scb_v = scb[:].rearrange("p (k x) -> p k x", x=2)
            nmin = mmp.tile([128, 1], f32, tag="nmin")
            gmax = mmp.tile([128, 1], f32, tag="gmax")
            nc.vector.tensor_reduce(nmin[:], scb_v[:, :, 0],
                                    mybir.AxisListType.X, alu.max)
            nc.vector.tensor_reduce(gmax[:], scb_v[:, :, 1],
                                    mybir.AxisListType.X, alu.max)
            rng = mmp.tile([128, 1], f32, tag="rng")
            nc.vector.tensor_tensor(rng[:], gmax[:], nmin[:], alu.add)
            inv = mmp.tile([128, 1], f32, tag="inv")
            nc.vector.reciprocal(inv[:], rng[:])
            off = mmp.tile([128, 1], f32, tag="off")
            nc.vector.tensor_tensor(off[:], nmin[:], inv[:], alu.mult)

            # ---- normalize + clip + store, per chunk ----
            for c in range(NCHUNK):
                oc = ostage[:, c * 2048: (c + 1) * 2048]
                nc.vector.tensor_scalar(oc, oc, inv[:], off[:],
                                        alu.mult, alu.add)
                nc.vector.tensor_scalar(oc, oc, 0.0, 1.0, alu.max, alu.min)
                for w in range(4):
                    for a in range(2):
                        p0 = 32 * w + 16 * a
                        nc.gpsimd.dma_start(
                            out=out_d[6 * w + 3 * a: 6 * w + 3 * a + 3,
                                      c * 2048: (c + 1) * 2048],
                            in_=ostage[p0: p0 + 3,
                                       c * 2048: (c + 1) * 2048])
    nc.compile()
    return nc


def _host_inputs(x, W1, b1, W2, b2, W3, b3, mm_np=None):
    """Repack full inputs into per-core in_maps (host-side, not HW-timed)."""
    if mm_np is None:
        import ml_dtypes
        mm = os.environ.get("CPPN_MM_DTYPE", "bfloat16")
        mm_np = ml_dtypes.bfloat16 if mm == "bfloat16" else np.float32
    x = np.asarray(x, np.float32).reshape(NPIX_TOT, NI)
    W1 = np.asarray(W1, np.float32)
    b1 = np.asarray(b1, np.float32)
    W2 = np.asarray(W2, np.float32)
    b2 = np.asarray(b2, np.float32)
    W3 = np.asarray(W3, np.float32)
    b3 = np.asarray(b3, np.float32)

    blk = np.zeros((32, 8 * H), np.float32)
    for q in range(8):
        blk[4 * q: 4 * q + 4, H * q: H * q + H] = W1
    w1s = np.tile(blk, (4, 1))

    w2s = np.concatenate([W2, W2], axis=0)
    w3bd = np.zeros((128, 32), np.float32)
    w3bd[0:64, 0:3] = W3        # a=0 half -> cols 0..2
    w3bd[64:128, 16:19] = W3    # a=1 half -> cols 16..18

    b1s = np.concatenate([b1, b1])[:, None].astype(np.float32)
    b2s = np.concatenate([b2, b2])[:, None].astype(np.float32)
    b3s = np.zeros((128, 1), np.float32)
    vms = np.zeros((128, 1), np.float32)
    nbs = np.full((128, 1), -1.0e30, np.float32)
    for p in range(128):
        if p % 16 < 3:
            b3s[p, 0] = b3[p % 16]
            vms[p, 0] = 1.0
            nbs[p, 0] = 0.0

    in_maps = []
    for k in range(NCORES):
        shard = x[k * NPIX: (k + 1) * NPIX].reshape(NSUB, SUBPIX, NI)
        xcore = np.empty((128, SUBPIX), np.float32)
        for s in range(NSUB):
            g, q = s % 4, s // 4
            xcore[32 * g + 4 * q: 32 * g + 4 * q + 4, :] = shard[s].T
        in_maps.append({
            "xcore": np.ascontiguousarray(xcore).astype(mm_np),
            "w1s": w1s.astype(mm_np), "w2s": w2s.astype(mm_np),
            "w3bd": w3bd.astype(mm_np),
            "b1s": b1s, "b2s": b2s, "b3s": b3s,
            "validm": vms, "negb": nbs,
        })
    return in_maps


def _unshard(core_outs):
    """[24, NCHUNK*2048] per core -> [NO, B, N] full output.

    Row j = 6w + 3a + o; col = c*2048 + 512*b + n.
    Subset s = 8b + 2w + (a ^ (u & 1)) where u = pixel-pair col index —
    the layer-2 diagonal packing swaps halves on odd column-halves v,
    and v = u & 1 with u = 2t + v... i.e. parity of the 512-col index
    within the pair-tile. Global: for tile t, half v: u = 2t+v, and the
    data at partition-half a corresponds to subset 4t + 2v + (a ^ v).
    Here (w, bkk) give u = 4*bkk + w within a chunk... inverted below.
    """
    out = np.empty((NO, NPIX_TOT), np.float32)
    for k in range(NCORES):
        arr = np.asarray(core_outs[k]).reshape(24, NCHUNK, 4, 512)
        for j in range(24):
            w, a, o = j // 6, (j % 6) // 3, j % 3
            for bkk in range(4):
                u = 4 * bkk + w          # within-chunk L3 emission index
                t, v = u // 2, u % 2
                s = 4 * t + 2 * v + (a ^ v)
                # pixels: s*4096 + c*512 + n  for c in 0..7, n in 0..511
                dst = out[o, k * NPIX + s * SUBPIX:
                          k * NPIX + (s + 1) * SUBPIX].reshape(NCHUNK, 512)
                dst[:, :] = arr[j, :, bkk, :]
    return out.reshape(NO, B, N)


def kernel(x, W1, b1, W2, b2, W3, b3):
    global LAST_RESULTS
    from concourse.bass_utils import run_bass_kernel_spmd

    mm = os.environ.get("CPPN_MM_DTYPE", "bfloat16")
    if mm not in _CACHE:
        _CACHE[mm] = _build_module(mm)
    nc = _CACHE[mm]

    in_maps = _host_inputs(x, W1, b1, W2, b2, W3, b3)
    res = run_bass_kernel_spmd(nc, in_maps, list(range(NCORES)))
    LAST_RESULTS = res
    return _unshard([res.results[k]["out"] for k in range(NCORES)])


# revision 35
# speedup vs baseline: 1.2853x; 1.2331x over previous
"""Trainium2 Bass kernel for nn_CPPN: 3-layer MLP (4->64->64->3, tanh) over
1M pixels + global min/max normalization, data-parallel over 8 NeuronCores.

Layout strategy (per core, NPIX = 131072 pixels):
  - pixels split into 32 "subsets" of 4096 contiguous pixels; subset s lives
    at partitions 32*(s%4) + 4*(s//4) + i (i = input feature), so layer-1
    runs as K=32 matmuls with zero-padded weights, 4 row-groups concurrent.
  - hidden states keep features on partitions ([64|64] per [128, 1024] tile
    = 4 subsets); tanh runs as full-128-lane ACT ops straight out of 2-bank
    PSUM tiles with the bias fused into the activation.
  - layer-2 packs 4 concurrent 64x64 matmuls via (row, col) tile positions,
    swapping output halves on odd column-halves (undone at host unshard).
  - layer-3 uses a block-diagonal [128, 32] weight emitting two subsets' 3
    channels at partition offsets {0,1,2,16,17,18} of a 32-slot, 4 slots
    per PSUM bank; DVE evacuates with fused bias.
  - ACT is software-pipelined (tanh2 of pair t emitted after tanh1 of
    t+1) with a single shared 3-slot PSUM pool so tanh runs back-to-back.
  - global min/max: per-partition running min/max on DVE, gpsimd
    cross-partition reduce, one tiny AllGather, then per-chunk
    normalize+clip+DMA-out interleaved.
"""

import os
import numpy as np

B, N, NI, H, NO = 4, 262144, 4, 64, 3
NCORES = 8
NPIX_TOT = B * N
NPIX = NPIX_TOT // NCORES      # 131072 pixels per core
NSUB = 32                      # subsets per core
SUBPIX = NPIX // NSUB          # 4096 pixels per subset
CW = 512                       # matmul moving-dim chunk width
NCHUNK = SUBPIX // CW          # 8 chunks
NT = NCHUNK * 8                # 64 pair-tiles total (8 per chunk)
F32MAX = 3.0e38

_CACHE = {}
LAST_RESULTS = None            # test.py reads exec_time_ns from here


def _build_module(mm_dtype_name="bfloat16"):
    import concourse.bass as bass
    import concourse.tile as tile
    from concourse import bacc, mybir
    from concourse.tile import add_dep_helper

    dt = mybir.dt
    alu = mybir.AluOpType
    act = mybir.ActivationFunctionType
    f32 = dt.float32
    mmdt = getattr(dt, mm_dtype_name)

    nc = bacc.Bacc("TRN2", target_bir_lowering=False, debug=False,
                   num_devices=NCORES)

    x_d = nc.dram_tensor("xcore", [128, SUBPIX], mmdt, kind="ExternalInput").ap()
    w1_d = nc.dram_tensor("w1s", [128, 8 * H], mmdt, kind="ExternalInput").ap()
    w2_d = nc.dram_tensor("w2s", [128, H], mmdt, kind="ExternalInput").ap()
    w3_d = nc.dram_tensor("w3bd", [128, 32], mmdt, kind="ExternalInput").ap()
    b1_d = nc.dram_tensor("b1s", [128, 1], f32, kind="ExternalInput").ap()
    b2_d = nc.dram_tensor("b2s", [128, 1], f32, kind="ExternalInput").ap()
    b3_d = nc.dram_tensor("b3s", [128, 1], f32, kind="ExternalInput").ap()
    vm_d = nc.dram_tensor("validm", [128, 1], f32, kind="ExternalInput").ap()
    nb_d = nc.dram_tensor("negb", [128, 1], f32, kind="ExternalInput").ap()
    out_d = nc.dram_tensor("out", [24, NCHUNK * 2048], f32,
                           kind="ExternalOutput").ap()

    cc_in = nc.dram_tensor("cc_in", [256], f32).ap()
    cc_out = nc.dram_tensor("cc_out", [256 * NCORES], f32,
                            addr_space="Shared").ap()

    with tile.TileContext(nc) as tc:
        with tc.tile_pool(name="const", bufs=1) as const, \
             tc.tile_pool(name="stage", bufs=1) as stage, \
             tc.tile_pool(name="hid", bufs=2) as hid, \
             tc.tile_pool(name="mm", bufs=1) as mmp, \
             tc.tile_pool(name="pmm", bufs=3, space="PSUM") as pmm, \
             tc.tile_pool(name="ps3", bufs=2, space="PSUM") as ps3:

            xin = const.tile([128, SUBPIX], mmdt, tag="xin")
            w1s = const.tile([128, 8 * H], mmdt, tag="w1s")
            w2s = const.tile([128, H], mmdt, tag="w2s")
            w3bd = const.tile([128, 32], mmdt, tag="w3bd")
            b1s = const.tile([128, 1], f32, tag="b1s")
            b2s = const.tile([128, 1], f32, tag="b2s")
            b3s = const.tile([128, 1], f32, tag="b3s")
            vms = const.tile([128, 1], f32, tag="vms")
            nbs = const.tile([128, 1], f32, tag="nbs")

            nc.sync.dma_start(out=w1s[:], in_=w1_d)
            nc.sync.dma_start(out=xin[:, 0:CW], in_=x_d[:, 0:CW])
            nc.sync.dma_start(out=b1s[:], in_=b1_d)
            nc.sync.dma_start(out=w2s[:], in_=w2_d)
            nc.sync.dma_start(out=w3bd[:], in_=w3_d)
            nc.sync.dma_start(out=b2s[:], in_=b2_d)
            nc.sync.dma_start(out=b3s[:], in_=b3_d)
            nc.sync.dma_start(out=vms[:], in_=vm_d)
            nc.sync.dma_start(out=nbs[:], in_=nb_d)
            for c in range(1, NCHUNK):
                nc.sync.dma_start(out=xin[:, c * CW: (c + 1) * CW],
                                  in_=x_d[:, c * CW: (c + 1) * CW])

            # staged pre-norm output: partition 32w + 16a + o,
            # free c*2048 + 512*b + n; (w, a, b) identify subsets.
            ostage = stage.tile([128, NCHUNK * 2048], f32, tag="ostage")

            rmin = mmp.tile([128, 1], f32, tag="rmin")
            rmax = mmp.tile([128, 1], f32, tag="rmax")
            nc.vector.memset(rmin[:], F32MAX)
            nc.vector.memset(rmax[:], -F32MAX)

            def emit_l1(t):
                c = t // 8
                p1 = pmm.tile([128, 2 * CW], f32, tag="pmm",
                              name=f"p1t{t}")
                for v in range(2):
                    for a in range(2):
                        s = 4 * (t % 8) + 2 * v + a
                        g, q = s % 4, s // 4
                        nc.tensor.matmul(
                            out=p1[64 * a: 64 * a + 64, CW * v: CW * v + CW],
                            lhsT=w1s[32 * g: 32 * g + 32, H * q: H * q + H],
                            rhs=xin[32 * g: 32 * g + 32,
                                    c * CW: (c + 1) * CW],
                            start=True, stop=True,
                            tile_position=(32 * g, 64 * a))
                return p1

            def emit_tanh1(t, p1):
                h1 = hid.tile([128, 2 * CW], mmdt, tag="h1")
                nc.scalar.activation(h1[:], p1[:], act.Tanh, bias=b1s[:])
                return h1

            def emit_l2(t, h1):
                p2 = pmm.tile([128, 2 * CW], f32, tag="pmm")
                for v in range(2):
                    for a in range(2):
                        # odd column-half swaps output halves so all four
                        # matmuls pack onto disjoint PE subarray quadrants
                        ao = a ^ (v & 1)
                        nc.tensor.matmul(
                            out=p2[64 * ao: 64 * ao + 64,
                                   CW * v: CW * v + CW],
                            lhsT=w2s[64 * a: 64 * a + 64, :],
                            rhs=h1[64 * a: 64 * a + 64,
                                   CW * v: CW * v + CW],
                            start=True, stop=True,
                            tile_position=(64 * a, 64 * ao))
                return p2

            def emit_tanh2(t, p2):
                h2 = hid.tile([128, 2 * CW], mmdt, tag="h2")
                nc.scalar.activation(h2[:], p2[:], act.Tanh, bias=b2s[:])
                return h2

            ps3_box = [None]

            def emit_l3(t, h2):
                c = t // 8
                for v in range(2):
                    u = 2 * t + v
                    w = u % 4
                    if w == 0:
                        ps3_box[0] = ps3.tile([128, CW], f32, tag="p3",
                                              name=f"p3t{u}")
                    p3 = ps3_box[0]
                    nc.tensor.matmul(
                        out=p3[32 * w: 32 * w + 32, :],
                        lhsT=w3bd[:],
                        rhs=h2[:, CW * v: CW * v + CW],
                        start=True, stop=True,
                        tile_position=(0, 32 * w))
                    if w == 3:
                        bkk = (u // 4) % 4
                        nc.vector.tensor_scalar(
                            ostage[:, c * 2048 + CW * bkk:
                                   c * 2048 + CW * bkk + CW],
                            p3[:], b3s[:], None, alu.add)

            def emit_chunk_minmax(c):
                oc = ostage[:, c * 2048: (c + 1) * 2048]
                cmin = mmp.tile([128, 1], f32, tag="cmin")
                cmax = mmp.tile([128, 1], f32, tag="cmax")
                nc.vector.tensor_reduce(cmin[:], oc, mybir.AxisListType.X,
                                        alu.min)
                nc.vector.tensor_reduce(cmax[:], oc, mybir.AxisListType.X,
                                        alu.max)
                nc.vector.tensor_tensor(rmin[:], rmin[:], cmin[:], alu.min)
                nc.vector.tensor_tensor(rmax[:], rmax[:], cmax[:], alu.max)

            # ---- software-pipelined main loop ----
            # PE static order: L1(t+1), L2(t), L3(t-1)  — L1 prefill first
            # ACT static order: tanh1(t), tanh2(t-1)    — back-to-back
            p1s, p2s = {0: emit_l1(0)}, {}
            for t in range(NT + 1):
                if t < NT:
                    h1 = emit_tanh1(t, p1s.pop(t))
                    if t + 1 < NT:
                        p1s[t + 1] = emit_l1(t + 1)
                    p2s[t] = emit_l2(t, h1)
                if t - 1 >= 0:
                    tp = t - 1
                    h2 = emit_tanh2(tp, p2s.pop(tp))
                    emit_l3(tp, h2)
                    if tp % 8 == 7:
                        emit_chunk_minmax(tp // 8)

            # ---- global min/max via AllGather ----
            mmt = mmp.tile([128, 2], f32, tag="mmt")
            nc.vector.tensor_scalar(mmt[:, 0:1], rmin[:], -1.0, None, alu.mult)
            nc.vector.tensor_copy(mmt[:, 1:2], rmax[:])
            # mask garbage partitions to -inf so they never win the max
            nc.vector.tensor_scalar(mmt[:], mmt[:], vms[:], nbs[:],
                                    alu.mult, alu.add)
            gd = nc.sync.dma_start(out=cc_in.rearrange("(p x) -> p x", x=2),
                                   in_=mmt[:])
            coll = nc.gpsimd.collective_compute(
                "AllGather", alu.bypass,
                replica_groups=[list(range(NCORES))],
                ins=[cc_in], outs=[cc_out])
            add_dep_helper(coll.ins, gd.ins, reason="gather before allgather")
            scb = mmp.tile([128, 256 * NCORES], f32, tag="scb")
            bd = nc.sync.dma_start(out=scb[:],
                                   in_=cc_out.partition_broadcast(128))
            add_dep_helper(bd.ins, coll.ins, reason="bcast after allgather")

            scb_v = scb[:].rearrange("p (k x) -> p k x", x=2)
            nmin = mmp.tile([128, 1], f32, tag="nmin")
            gmax = mmp.tile([128, 1], f32, tag="gmax")
            nc.vector.tensor_reduce(nmin[:], scb_v[:, :, 0],
                                    mybir.AxisListType.X, alu.max)
            nc.vector.tensor_reduce(gmax[:], scb_v[:, :, 1],
                                    mybir.AxisListType.X, alu.max)
            rng = mmp.tile([128, 1], f32, tag="rng")
            nc.vector.tensor_tensor(rng[:], gmax[:], nmin[:], alu.add)
            inv = mmp.tile([128, 1], f32, tag="inv")
            nc.vector.reciprocal(inv[:], rng[:])
            off = mmp.tile([128, 1], f32, tag="off")
            nc.vector.tensor_tensor(off[:], nmin[:], inv[:], alu.mult)

            # ---- normalize + clip + store, per chunk ----
            for c in range(NCHUNK):
                oc = ostage[:, c * 2048: (c + 1) * 2048]
                nc.vector.tensor_scalar(oc, oc, inv[:], off[:],
                                        alu.mult, alu.add)
                nc.vector.tensor_scalar(oc, oc, 0.0, 1.0, alu.max, alu.min)
                for w in range(4):
                    for a in range(2):
                        p0 = 32 * w + 16 * a
                        nc.sync.dma_start(
                            out=out_d[6 * w + 3 * a: 6 * w + 3 * a + 3,
                                      c * 2048: (c + 1) * 2048],
                            in_=ostage[p0: p0 + 3,
                                       c * 2048: (c + 1) * 2048])
    nc.compile()
    return nc


def _host_inputs(x, W1, b1, W2, b2, W3, b3, mm_np=None):
    """Repack full inputs into per-core in_maps (host-side, not HW-timed)."""
    if mm_np is None:
        import ml_dtypes
        mm = os.environ.get("CPPN_MM_DTYPE", "bfloat16")
        mm_np = ml_dtypes.bfloat16 if mm == "bfloat16" else np.float32
    x = np.asarray(x, np.float32).reshape(NPIX_TOT, NI)
    W1 = np.asarray(W1, np.float32)
    b1 = np.asarray(b1, np.float32)
    W2 = np.asarray(W2, np.float32)
    b2 = np.asarray(b2, np.float32)
    W3 = np.asarray(W3, np.float32)
    b3 = np.asarray(b3, np.float32)

    blk = np.zeros((32, 8 * H), np.float32)
    for q in range(8):
        blk[4 * q: 4 * q + 4, H * q: H * q + H] = W1
    w1s = np.tile(blk, (4, 1))

    w2s = np.concatenate([W2, W2], axis=0)
    w3bd = np.zeros((128, 32), np.float32)
    w3bd[0:64, 0:3] = W3        # a=0 half -> cols 0..2
    w3bd[64:128, 16:19] = W3    # a=1 half -> cols 16..18

    b1s = np.concatenate([b1, b1])[:, None].astype(np.float32)
    b2s = np.concatenate([b2, b2])[:, None].astype(np.float32)
    b3s = np.zeros((128, 1), np.float32)
    vms = np.zeros((128, 1), np.float32)
    nbs = np.full((128, 1), -1.0e30, np.float32)
    for p in range(128):
        if p % 16 < 3:
            b3s[p, 0] = b3[p % 16]
            vms[p, 0] = 1.0
            nbs[p, 0] = 0.0

    in_maps = []
    for k in range(NCORES):
        shard = x[k * NPIX: (k + 1) * NPIX].reshape(NSUB, SUBPIX, NI)
        xcore = np.empty((128, SUBPIX), np.float32)
        for s in range(NSUB):
            g, q = s % 4, s // 4
            xcore[32 * g + 4 * q: 32 * g + 4 * q + 4, :] = shard[s].T
        in_maps.append({
            "xcore": np.ascontiguousarray(xcore).astype(mm_np),
            "w1s": w1s.astype(mm_np), "w2s": w2s.astype(mm_np),
            "w3bd": w3bd.astype(mm_np),
            "b1s": b1s, "b2s": b2s, "b3s": b3s,
            "validm": vms, "negb": nbs,
        })
    return in_maps


def _unshard(core_outs):
    """[24, NCHUNK*2048] per core -> [NO, B, N] full output.

    Row j = 6w + 3a + o; col = c*2048 + 512*b + n.
    Subset s = 8b + 2w + (a ^ (u & 1)) where u = pixel-pair col index —
    the layer-2 diagonal packing swaps halves on odd column-halves v,
    and v = u & 1 with u = 2t + v... i.e. parity of the 512-col index
    within the pair-tile. Global: for tile t, half v: u = 2t+v, and the
    data at partition-half a corresponds to subset 4t + 2v + (a ^ v).
    Here (w, bkk) give u = 4*bkk + w within a chunk... inverted below.
    """
    out = np.empty((NO, NPIX_TOT), np.float32)
    for k in range(NCORES):
        arr = np.asarray(core_outs[k]).reshape(24, NCHUNK, 4, 512)
        for j in range(24):
            w, a, o = j // 6, (j % 6) // 3, j % 3
            for bkk in range(4):
                u = 4 * bkk + w          # within-chunk L3 emission index
                t, v = u // 2, u % 2
                s = 4 * t + 2 * v + (a ^ v)
                # pixels: s*4096 + c*512 + n  for c in 0..7, n in 0..511
                dst = out[o, k * NPIX + s * SUBPIX:
                          k * NPIX + (s + 1) * SUBPIX].reshape(NCHUNK, 512)
                dst[:, :] = arr[j, :, bkk, :]
    return out.reshape(NO, B, N)


def kernel(x, W1, b1, W2, b2, W3, b3):
    global LAST_RESULTS
    from concourse.bass_utils import run_bass_kernel_spmd

    mm = os.environ.get("CPPN_MM_DTYPE", "bfloat16")
    if mm not in _CACHE:
        _CACHE[mm] = _build_module(mm)
    nc = _CACHE[mm]

    in_maps = _host_inputs(x, W1, b1, W2, b2, W3, b3)
    res = run_bass_kernel_spmd(nc, in_maps, list(range(NCORES)))
    LAST_RESULTS = res
    return _unshard([res.results[k]["out"] for k in range(NCORES)])
